# revision 29
# baseline (speedup 1.0000x reference)
"""Bias multi-head attention kernel for Trainium2 (8 NeuronCores).

Problem: x[B=4,N=2048,D=1024], 16 heads, dh=64; attn bias (scaled by
beta) added to the first 8 heads; qkv proj -> attention -> out proj.

Sharding: core = (b, parity); b = core//2, parity = core%2. Each core
handles batch b and the 8 heads hlist = [parity, parity+2, ...] (the 4
biased heads h<8 plus 4 unbiased). Each core computes the partial
y_b = sum over its heads; the host adds the two parity cores per batch.

Per-core device pipeline (matmul operands bf16 under CONFIG[all_bf16]
— halves the serial weight-load cost of self-loading matmuls and the
HBM traffic; PSUM accumulation is always fp32):
  phase 1: QKV. q^T, k^T head-pairs and V (with an appended ones
           column) are made SBUF-resident, q/k UNSCALED (the 1/sqrt(dh)
           scale is folded into the exp). PSUM->SBUF copies alternate
           ACT/DVE; the QKV weight tiles are loaded once (w_const).
  phase 2: attention, software-pipelined per (j=head pair, m=key tile):
           S^T pair tile via K^T.T @ q^T (two heads in PE row-groups
           0-63/64-127, one 2-bank PSUM tile); for biased pairs the
           bias lands IN the same PSUM accumulation group via an
           identity matmul (stationary = 8*beta_h*I, moving = bias^T
           tile) — no vector-engine bias op, no extra sync hop; one
           paired exp on ACT (scale=0.125); P^T @ [V|1] accumulates
           over m two steps behind the logits (PSUM rows 0-63 = o^T,
           row 64 = softmax denominator). Normalize: DVE reciprocal,
           partition-broadcast via a 1-row PE matmul (ones^T @ recip)
           into PSUM, DVE copy+multiply. Out-proj accumulates head
           pairs (K=128) into lp-pool PSUM slots, copy, DMA out.

PSUM plan (8 banks): 2x [128,1024] logits/proj slots (4) + 3x [65,512]
PV accumulators + 1x [64,512] broadcast. Bias tiles for the first two
n-blocks prefetch during phase 1 (33-deep pool); x tiles are fully
double-buffered across the rep. Timing A/Bs on HW: f32r 823us (prior
baseline) -> 646 (engine rebalance + resident q) -> ~555 (bf16
operands + const weights + deep prefetch). Per-matmul weight-load
serialization (~70ns f32r, ~35ns bf16, measured) is the main residual
over the ~390us PE-cycle floor.
"""

import numpy as np
import concourse.bass as bass
import concourse.mybir as mybir
import concourse.tile as tile

f32 = mybir.dt.float32
f32r = mybir.dt.float32r
bf16 = mybir.dt.bfloat16
AF = mybir.ActivationFunctionType
ALU = mybir.AluOpType

N = 2048
D = 1024
NB = 512
NBLK = N // NB
MT = N // 128
KD = D // 128
SCALE = 0.125


def _split_multi_waits(nc, limit=1):
    """This walrus build rejects >1 sync wait per instruction; hoist
    extra waits onto standalone same-engine NoOps placed before it."""
    n_split = 0
    for f in nc.m.functions:
        for bb in f.blocks:
            new_insts = []
            for inst in bb.instructions:
                si = inst.sync_info
                waits = list(si.on_wait) if si is not None and si.on_wait else []
                if len(waits) > limit:
                    extra, keep = waits[:-limit], waits[-limit:]
                    for i in range(0, len(extra), limit):
                        nop = mybir.InstNoOp(
                            name=f"{inst.name}.wsplit{i}", ins=[], outs=[]
                        )
                        nop.engine = inst.engine
                        nop.sync_info = mybir.SyncInfo(
                            on_wait=extra[i : i + limit], on_update=[]
                        )
                        nc.register_instruction(nop, overwrite=True)
                        new_insts.append(nop)
                        n_split += 1
                    inst.sync_info = mybir.SyncInfo(
                        on_wait=keep, on_update=list(si.on_update or [])
                    )
                new_insts.append(inst)
            bb.instructions = new_insts
    return n_split


def build_nc(reps=1, biased_pairs=2, ysb_mode="act", bcs_mode="dve", dve_bias_heads=0, timing=False, w_const=False, attn_bf16=False, all_bf16=False, proj_split=True, norm_delay=True):
    # timing=True: no ExternalInputs and a 16-byte output, so a timing run
    # transfers (almost) nothing over the axon tunnel; the kernel body is
    # identical, reading whatever garbage sits in device DRAM.
    nc = bass.Bass("TRN2", debug=False)
    if all_bf16:
        attn_bf16 = True
    io_dt = bf16 if all_bf16 else f32r
    kw = {} if timing else {"kind": "ExternalInput"}
    xT = nc.dram_tensor("xT", [D, N], io_dt, **kw)
    wqkvT = nc.dram_tensor("wqkvT", [D, 1536], io_dt, **kw)
    wprojT = nc.dram_tensor("wprojT", [512, 1024], io_dt, **kw)
    biasT = nc.dram_tensor("biasT", [N, N], io_dt, **kw)
    ident4 = nc.dram_tensor("ident4", [128, 512], io_dt, **kw)
    beta8 = nc.dram_tensor("beta8", [128, 4], f32, **kw)
    if timing:
        y = nc.dram_tensor("y", [N, D], f32)
        ydum = nc.dram_tensor("ydum", [1, 4], f32, kind="ExternalOutput")
    else:
        y = nc.dram_tensor("y", [N, D], f32, kind="ExternalOutput")

    qk_dt = bf16 if attn_bf16 else f32r

    with tile.TileContext(nc) as tc:
        with (
            tc.tile_pool(name="const", bufs=1) as const_pool,
            tc.tile_pool(name="kvres", bufs=1) as kvres,
            tc.tile_pool(name="wproj", bufs=1) as wproj_pool,
        ):
            ones_row = const_pool.tile([1, 128], f32r, tag="ones")
            nc.vector.memset(ones_row[:].bitcast(f32), 1.0)
            ident_sb = const_pool.tile([128, 512], io_dt, tag="ident")
            nc.sync.dma_start(out=ident_sb[:], in_=ident4[:])
            beta8_sb = const_pool.tile([128, 4], f32, tag="beta8")
            nc.sync.dma_start(out=beta8_sb[:], in_=beta8[:])

            kT = [kvres.tile([128, N], qk_dt, tag=f"kT{j}", name=f"kT{j}") for j in range(4)]
            qT = [kvres.tile([128, N], qk_dt, tag=f"qT{j}", name=f"qT{j}") for j in range(4)]
            V = [kvres.tile([128, 8 * 65], qk_dt, tag=f"V{m}", name=f"V{m}") for m in range(MT)]
            wproj_sb = [
                wproj_pool.tile([128, 1024], io_dt, tag=f"wp{j}", name=f"wp{j}")
                for j in range(4)
            ]
            for j in range(4):
                nc.sync.dma_start(
                    out=wproj_sb[j][:], in_=wprojT[j * 128 : (j + 1) * 128, :]
                )
            for m in range(MT):
                ones_col = V[m][:].rearrange("p (h c) -> p h c", c=65)[:, :, 64:65]
                if not attn_bf16:
                    ones_col = ones_col.bitcast(f32)
                nc.vector.memset(ones_col, 1.0)

            w_const_sb = None
            if w_const:
                w_const_sb = []
                for k in range(KD):
                    wt = kvres.tile([128, 1536], io_dt, tag=f"wc{k}", name=f"wc{k}")
                    nc.sync.dma_start(
                        out=wt[:], in_=wqkvT[k * 128 : (k + 1) * 128, :]
                    )
                    w_const_sb.append(wt)

            def body(_=None):
                bias_ctx = tc.tile_pool(name="bias", bufs=33)
                bias_pool = bias_ctx.__enter__()

                def load_bias(nb):
                    tiles = []
                    for m in range(MT):
                        bt = bias_pool.tile([128, NB], io_dt, tag="bias")
                        nc.sync.dma_start(
                            out=bt[:],
                            in_=biasT[
                                m * 128 : (m + 1) * 128, nb * NB : (nb + 1) * NB
                            ],
                        )
                        tiles.append(bt)
                    return tiles

                bias_tiles = {}
                # phase 1: QKV -> resident qT, kT, V
                with (
                    tc.tile_pool(name="wqkv", bufs=1 if w_const else KD) as wpool,
                    tc.tile_pool(name="xs", bufs=32) as xpool,
                    tc.tile_pool(name="qk_ps", bufs=5, space="PSUM") as qk_ps,
                    tc.tile_pool(name="v_ps", bufs=3, space="PSUM") as v_ps,
                ):
                    if w_const:
                        w_sb = w_const_sb
                    else:
                        w_sb = []
                        for k in range(KD):
                            wt = wpool.tile([128, 1536], io_dt, tag="w")
                            nc.sync.dma_start(
                                out=wt[:], in_=wqkvT[k * 128 : (k + 1) * 128, :]
                            )
                            w_sb.append(wt)
                    copy_engines = [
                        lambda o, i: nc.scalar.copy(o, i),
                        lambda o, i: nc.vector.tensor_copy(o, i),
                    ]
                    ncopy = 0
                    for nb in range(NBLK):
                        x_sb = []
                        for k in range(KD):
                            xt = xpool.tile([128, NB], io_dt, tag="x")
                            nc.sync.dma_start(
                                out=xt[:],
                                in_=xT[k * 128 : (k + 1) * 128, nb * NB : (nb + 1) * NB],
                            )
                            x_sb.append(xt)
                        if nb == 0:
                            bias_tiles[0] = load_bias(0)
                            bias_tiles[1] = load_bias(1)
                        for e in range(8):
                            ps = qk_ps.tile([128, NB], f32, tag="qk")
                            for k in range(KD):
                                nc.tensor.matmul(
                                    ps[:],
                                    w_sb[k][:, e * 128 : (e + 1) * 128],
                                    x_sb[k][:],
                                    start=(k == 0),
                                    stop=(k == KD - 1),
                                )
                            dst = qT[e] if e < 4 else kT[e - 4]
                            eng = copy_engines[ncopy % 2]
                            ncopy += 1
                            eng(dst[:, nb * NB : (nb + 1) * NB], ps[:])
                        for mi in range(4):
                            m = nb * 4 + mi
                            ps = v_ps.tile([128, NB], f32, tag="v")
                            for k in range(KD):
                                nc.tensor.matmul(
                                    ps[:],
                                    x_sb[k][:, mi * 128 : (mi + 1) * 128],
                                    w_sb[k][:, 1024:1536],
                                    start=(k == 0),
                                    stop=(k == KD - 1),
                                )
                            eng = copy_engines[ncopy % 2]
                            ncopy += 1
                            eng(
                                V[m][:].rearrange("p (h c) -> p h c", c=65)[:, :, 0:64],
                                ps[:].rearrange("p (h c) -> p h c", c=64),
                            )

                # phase 2: attention + projection
                with (
                    tc.tile_pool(name="esb", bufs=8) as e_pool,
                    tc.tile_pool(name="opair", bufs=8) as o_pool,
                    tc.tile_pool(name="ysb", bufs=3) as y_pool,
                    tc.tile_pool(name="nrm", bufs=4) as nrm_pool,
                    tc.tile_pool(name="l_ps", bufs=2, space="PSUM") as l_ps,
                    tc.tile_pool(name="o_ps", bufs=3, space="PSUM") as o_psp,
                    tc.tile_pool(name="bc_ps", bufs=1, space="PSUM") as bc_psp,
                ):
                    o_pairs = [None] * 4
                    for nb in range(NBLK):
                        bias_sb = bias_tiles.pop(nb)
                        o_ps = [None] * 4
                        pend = []
                        norm_pend = []

                        def recip_stage(j):
                            recips = []
                            for hh in range(2):
                                recip = nrm_pool.tile([1, NB], f32r, tag="recip")
                                with nc.allow_low_precision(
                                    reason="f32r recip feeds f32r matmul"
                                ):
                                    nc.vector.reciprocal(
                                        recip[:], o_ps[j][hh][64:65, :]
                                    )
                                recips.append(recip)
                            return recips

                        def norm_stage(j, recips):
                            o_pair = o_pool.tile([128, NB], io_dt, tag="op")
                            o_pairs[j] = o_pair
                            for hh in range(2):
                                off = hh * 64
                                bc = bc_psp.tile([64, NB], f32, tag="bc")
                                nc.tensor.matmul(
                                    bc[:],
                                    ones_row[:, 0:64],
                                    recips[hh][:],
                                    start=True,
                                    stop=True,
                                )
                                bcs = nrm_pool.tile([64, NB], f32, tag="bcs")
                                if bcs_mode == "act":
                                    nc.scalar.copy(bcs[:], bc[:])
                                else:
                                    nc.vector.tensor_copy(bcs[:], bc[:])
                                nc.vector.tensor_tensor(
                                    o_pair[off : off + 64, :],
                                    o_ps[j][hh][0:64, :],
                                    bcs[:],
                                    ALU.mult,
                                )

                        def emit_pv(j, m, et):
                            for hh in range(2):
                                h = 2 * j + hh
                                nc.tensor.matmul(
                                    o_ps[j][hh][:],
                                    V[m][:, h * 65 : (h + 1) * 65],
                                    et[:, hh * NB : (hh + 1) * NB],
                                    start=(m == 0),
                                    stop=(m == MT - 1),
                                )
                            if m == MT - 1:
                                norm_pend.append((j, recip_stage(j)))
                                if not norm_delay:
                                    flush_norms()

                        def flush_norms():
                            while norm_pend:
                                norm_stage(*norm_pend.pop(0))

                        for j in range(4):
                            biased = j < biased_pairs
                            o_ps[j] = [
                                o_psp.tile([65, NB], f32, tag="ops", name=f"o{nb}_{j}_{hh}")
                                for hh in range(2)
                            ]
                            for m in range(MT):
                                lp = l_ps.tile([128, 2 * NB], f32, tag="lp")
                                for hh in range(2):
                                    off = hh * 64
                                    i = 2 * j + hh
                                    on_dve = biased and i < dve_bias_heads
                                    nc.tensor.matmul(
                                        lp[:, hh * NB : (hh + 1) * NB],
                                        kT[j][off : off + 64, m * 128 : (m + 1) * 128],
                                        qT[j][off : off + 64, nb * NB : (nb + 1) * NB],
                                        start=True,
                                        stop=(not biased) or on_dve,
                                        tile_position=(off, 0),
                                    )
                                if biased:
                                    for hh in range(2):
                                        i = 2 * j + hh
                                        if i < dve_bias_heads:
                                            nc.vector.scalar_tensor_tensor(
                                                lp[:, hh * NB : (hh + 1) * NB],
                                                bias_sb[m][:].bitcast(f32),
                                                beta8_sb[:, i : i + 1],
                                                lp[:, hh * NB : (hh + 1) * NB],
                                                op0=ALU.mult,
                                                op1=ALU.add,
                                            )
                                        else:
                                            nc.tensor.matmul(
                                                lp[:, hh * NB : (hh + 1) * NB],
                                                ident_sb[:, i * 128 : (i + 1) * 128],
                                                bias_sb[m][:],
                                                start=False,
                                                stop=True,
                                            )
                                et = e_pool.tile([128, 2 * NB], qk_dt, tag="e")
                                nc.scalar.activation(et[:], lp[:], AF.Exp, scale=SCALE)
                                pend.append((j, m, et))
                                if len(pend) > 2:
                                    emit_pv(*pend.pop(0))
                                    if norm_pend and len(pend) > 1:
                                        flush_norms()
                            if j == 1 and nb + 2 < NBLK:
                                bias_tiles[nb + 2] = load_bias(nb + 2)
                        while pend:
                            emit_pv(*pend.pop(0))
                        flush_norms()

                        for pair in ((0, 1), (2, 3)):
                            yps = {}
                            nj = 3 if proj_split else 4
                            for nt in pair:
                                yp = l_ps.tile([128, 2 * NB], f32, tag="lp")
                                yps[nt] = yp
                                for db in range(2):
                                    for j in range(nj):
                                        nc.tensor.matmul(
                                            yp[:, db * NB : (db + 1) * NB],
                                            o_pairs[j][:, nt * 128 : (nt + 1) * 128],
                                            wproj_sb[j][:, db * NB : (db + 1) * NB],
                                            start=(j == 0),
                                            stop=(j == 3),
                                        )
                            for nt in pair:
                                yp = yps[nt]
                                if proj_split:
                                    for db in range(2):
                                        nc.tensor.matmul(
                                            yp[:, db * NB : (db + 1) * NB],
                                            o_pairs[3][:, nt * 128 : (nt + 1) * 128],
                                            wproj_sb[3][:, db * NB : (db + 1) * NB],
                                            start=False,
                                            stop=True,
                                        )
                                ysb = y_pool.tile([128, 2 * NB], f32, tag="y")
                                if ysb_mode == "act":
                                    nc.scalar.copy(ysb[:], yp[:])
                                else:
                                    nc.vector.tensor_copy(ysb[:], yp[:])
                                nc.sync.dma_start(
                                    out=y[
                                        nb * NB + nt * 128 : nb * NB + (nt + 1) * 128, :
                                    ],
                                    in_=ysb[:],
                                )

                bias_ctx.__exit__(None, None, None)

            if reps == 1:
                body()
            else:
                with tc.For_i(0, reps, 1):
                    body()
            if timing:
                dum = const_pool.tile([1, 4], f32, tag="dum")
                nc.vector.memset(dum[:], 0.0)
                nc.sync.dma_start(out=ydum[:], in_=dum[:])

    _split_multi_waits(nc)
    nc.finalize()
    return nc


def make_core_inputs(x, attn_bias, Wqkv, Wproj, beta, core_id, all_bf16=False):
    import ml_dtypes
    io_np = ml_dtypes.bfloat16 if all_bf16 else np.float32
    b = core_id // 2
    parity = core_id % 2
    hlist = list(range(parity, 16, 2))
    rows = np.concatenate([np.arange(h * 64, (h + 1) * 64) for h in hlist])
    wqkvT = np.ascontiguousarray(
        np.concatenate([Wqkv[rows], Wqkv[D + rows], Wqkv[2 * D + rows]], 0).T
    )
    wprojT = np.ascontiguousarray(Wproj.T[rows])
    beta4 = np.asarray(beta).reshape(-1)[hlist[:4]].astype(np.float32)
    ident4 = np.zeros((128, 512), dtype=np.float32)
    eye = np.eye(128, dtype=np.float32)
    for i in range(4):
        # exp is taken with scale=0.125, so pre-scale the bias by 8*beta
        ident4[:, i * 128 : (i + 1) * 128] = eye * (8.0 * beta4[i])
    return {
        "xT": np.ascontiguousarray(x[b].T).astype(io_np),
        "wqkvT": wqkvT.astype(io_np),
        "wprojT": wprojT.astype(io_np),
        "biasT": np.ascontiguousarray(attn_bias[b, 0].T).astype(io_np),
        "ident4": ident4.astype(io_np),
        "beta8": np.tile(8.0 * beta4.reshape(1, 4), (128, 1)).astype(np.float32),
    }


_NC_CACHE = {}

# Graded configuration (HW A/Bs in test.py): bf16 matmul operands
# everywhere (fp32 PSUM), QKV weights resident across reps, split
# out-proj accumulation, inline normalize emission.
# Measured: 553.8us/iter, absmax_rel=1.009e-02 (gate 2e-2).
CONFIG = dict(attn_bf16=True, all_bf16=True, w_const=True,
              proj_split=True, norm_delay=False)


def kernel(x, attn_bias, Wqkv, Wproj, beta):
    from concourse.bass_utils import run_bass_kernel_spmd

    x = np.asarray(x, dtype=np.float32)
    attn_bias = np.asarray(attn_bias, dtype=np.float32)
    Wqkv = np.asarray(Wqkv, dtype=np.float32)
    Wproj = np.asarray(Wproj, dtype=np.float32)
    beta = np.asarray(beta, dtype=np.float32)

    key = tuple(sorted(CONFIG.items()))
    if key not in _NC_CACHE:
        _NC_CACHE[key] = build_nc(reps=1, **CONFIG)
    nc = _NC_CACHE[key]

    in_maps = [
        make_core_inputs(
            x, attn_bias, Wqkv, Wproj, beta, core, all_bf16=CONFIG["all_bf16"]
        )
        for core in range(8)
    ]
    res = run_bass_kernel_spmd(nc, in_maps, core_ids=list(range(8)))
    out = np.zeros((4, N, D), dtype=np.float32)
    for core_id in range(8):
        out[core_id // 2] += res.results[core_id]["y"]
    return out


# revision 32
# speedup vs baseline: 1.2052x; 1.2052x over previous
"""Bias multi-head attention kernel for Trainium2 (8 NeuronCores).

Problem: x[B=4,N=2048,D=1024], 16 heads, dh=64; attn bias (scaled by
beta) added to the first 8 heads; qkv proj -> attention -> out proj.

Sharding: core = (b, parity); b = core//2, parity = core%2. Each core
handles batch b and the 8 heads hlist = [parity, parity+2, ...] (the 4
biased heads h<8 plus 4 unbiased). Each core computes the partial
y_b = sum over its heads; the host adds the two parity cores per batch.

Per-core device pipeline (matmul operands bf16 under CONFIG[all_bf16]
— halves the serial weight-load cost of self-loading matmuls and the
HBM traffic; PSUM accumulation is always fp32):
  phase 1: QKV. q^T, k^T head-pairs and V (with an appended ones
           column) are made SBUF-resident, q/k UNSCALED (the 1/sqrt(dh)
           scale is folded into the exp). PSUM->SBUF copies alternate
           ACT/DVE; the QKV weight tiles are loaded once (w_const).
  phase 2: attention, software-pipelined per (j=head pair, m=key tile):
           S^T pair tile via K^T.T @ q^T (two heads in PE row-groups
           0-63/64-127, one 2-bank PSUM tile); for biased pairs the
           bias lands IN the same PSUM accumulation group via an
           identity matmul (stationary = 8*beta_h*I, moving = bias^T
           tile) — no vector-engine bias op, no extra sync hop; one
           paired exp on ACT (scale=0.125); P^T @ [V|1] accumulates
           over m two steps behind the logits (PSUM rows 0-63 = o^T,
           row 64 = softmax denominator). Normalize: DVE reciprocal,
           partition-broadcast via a 1-row PE matmul (ones^T @ recip)
           into PSUM, DVE copy+multiply. Out-proj accumulates head
           pairs (K=128) into lp-pool PSUM slots, copy, DMA out.

PSUM plan (8 banks): 2x [128,1024] logits/proj slots (4) + 3x [65,512]
PV accumulators + 1x [64,512] broadcast. Bias tiles for the first two
n-blocks prefetch during phase 1 (33-deep pool); x tiles are fully
double-buffered across the rep; the logits/PV pipeline is primed two
steps into the next n-block before each projection so the normalize
chain and proj overlap real PE work (no nb-boundary stall). Timing
A/Bs on HW: f32r 823us (prior baseline) -> 646 (engine rebalance +
resident q) -> ~555-587 (bf16 operands + const weights + deep
prefetch + cross-nb priming; cross-session drift ~5-15%). Per-matmul
weight-load serialization (~70ns f32r, ~35ns bf16, measured) is the
main residual over the ~390us PE-cycle floor; verified absmax_rel =
1.009e-02 vs the 2e-2 gate.
"""

import numpy as np
import concourse.bass as bass
import concourse.mybir as mybir
import concourse.tile as tile

f32 = mybir.dt.float32
f32r = mybir.dt.float32r
bf16 = mybir.dt.bfloat16
AF = mybir.ActivationFunctionType
ALU = mybir.AluOpType

N = 2048
D = 1024
NB = 512
NBLK = N // NB
MT = N // 128
KD = D // 128
SCALE = 0.125


def _split_multi_waits(nc, limit=1):
    """This walrus build rejects >1 sync wait per instruction; hoist
    extra waits onto standalone same-engine NoOps placed before it."""
    n_split = 0
    for f in nc.m.functions:
        for bb in f.blocks:
            new_insts = []
            for inst in bb.instructions:
                si = inst.sync_info
                waits = list(si.on_wait) if si is not None and si.on_wait else []
                if len(waits) > limit:
                    extra, keep = waits[:-limit], waits[-limit:]
                    for i in range(0, len(extra), limit):
                        nop = mybir.InstNoOp(
                            name=f"{inst.name}.wsplit{i}", ins=[], outs=[]
                        )
                        nop.engine = inst.engine
                        nop.sync_info = mybir.SyncInfo(
                            on_wait=extra[i : i + limit], on_update=[]
                        )
                        nc.register_instruction(nop, overwrite=True)
                        new_insts.append(nop)
                        n_split += 1
                    inst.sync_info = mybir.SyncInfo(
                        on_wait=keep, on_update=list(si.on_update or [])
                    )
                new_insts.append(inst)
            bb.instructions = new_insts
    return n_split


def build_nc(reps=1, biased_pairs=2, ysb_mode="act", bcs_mode="dve", dve_bias_heads=0, timing=False, w_const=False, attn_bf16=False, all_bf16=False, proj_split=True, norm_delay=True):
    # timing=True: no ExternalInputs and a 16-byte output, so a timing run
    # transfers (almost) nothing over the axon tunnel; the kernel body is
    # identical, reading whatever garbage sits in device DRAM.
    nc = bass.Bass("TRN2", debug=False)
    if all_bf16:
        attn_bf16 = True
    io_dt = bf16 if all_bf16 else f32r
    kw = {} if timing else {"kind": "ExternalInput"}
    xT = nc.dram_tensor("xT", [D, N], io_dt, **kw)
    wqkvT = nc.dram_tensor("wqkvT", [D, 1536], io_dt, **kw)
    wprojT = nc.dram_tensor("wprojT", [512, 1024], io_dt, **kw)
    biasT = nc.dram_tensor("biasT", [N, N], io_dt, **kw)
    ident4 = nc.dram_tensor("ident4", [128, 512], io_dt, **kw)
    beta8 = nc.dram_tensor("beta8", [128, 4], f32, **kw)
    if timing:
        y = nc.dram_tensor("y", [N, D], f32)
        ydum = nc.dram_tensor("ydum", [1, 4], f32, kind="ExternalOutput")
    else:
        y = nc.dram_tensor("y", [N, D], f32, kind="ExternalOutput")

    qk_dt = bf16 if attn_bf16 else f32r

    with tile.TileContext(nc) as tc:
        with (
            tc.tile_pool(name="const", bufs=1) as const_pool,
            tc.tile_pool(name="kvres", bufs=1) as kvres,
            tc.tile_pool(name="wproj", bufs=1) as wproj_pool,
        ):
            ones_row = const_pool.tile([1, 128], f32r, tag="ones")
            nc.vector.memset(ones_row[:].bitcast(f32), 1.0)
            ident_sb = const_pool.tile([128, 512], io_dt, tag="ident")
            nc.sync.dma_start(out=ident_sb[:], in_=ident4[:])
            beta8_sb = const_pool.tile([128, 4], f32, tag="beta8")
            nc.sync.dma_start(out=beta8_sb[:], in_=beta8[:])

            kT = [kvres.tile([128, N], qk_dt, tag=f"kT{j}", name=f"kT{j}") for j in range(4)]
            qT = [kvres.tile([128, N], qk_dt, tag=f"qT{j}", name=f"qT{j}") for j in range(4)]
            V = [kvres.tile([128, 8 * 65], qk_dt, tag=f"V{m}", name=f"V{m}") for m in range(MT)]
            wproj_sb = [
                wproj_pool.tile([128, 1024], io_dt, tag=f"wp{j}", name=f"wp{j}")
                for j in range(4)
            ]
            for j in range(4):
                nc.sync.dma_start(
                    out=wproj_sb[j][:], in_=wprojT[j * 128 : (j + 1) * 128, :]
                )
            for m in range(MT):
                ones_col = V[m][:].rearrange("p (h c) -> p h c", c=65)[:, :, 64:65]
                if not attn_bf16:
                    ones_col = ones_col.bitcast(f32)
                nc.vector.memset(ones_col, 1.0)

            w_const_sb = None
            if w_const:
                w_const_sb = []
                for k in range(KD):
                    wt = kvres.tile([128, 1536], io_dt, tag=f"wc{k}", name=f"wc{k}")
                    nc.sync.dma_start(
                        out=wt[:], in_=wqkvT[k * 128 : (k + 1) * 128, :]
                    )
                    w_const_sb.append(wt)

            def body(_=None):
                bias_ctx = tc.tile_pool(name="bias", bufs=33)
                bias_pool = bias_ctx.__enter__()

                def load_bias(nb):
                    tiles = []
                    for m in range(MT):
                        bt = bias_pool.tile([128, NB], io_dt, tag="bias")
                        nc.sync.dma_start(
                            out=bt[:],
                            in_=biasT[
                                m * 128 : (m + 1) * 128, nb * NB : (nb + 1) * NB
                            ],
                        )
                        tiles.append(bt)
                    return tiles

                bias_tiles = {}
                # phase 1: QKV -> resident qT, kT, V
                with (
                    tc.tile_pool(name="wqkv", bufs=1 if w_const else KD) as wpool,
                    tc.tile_pool(name="xs", bufs=32) as xpool,
                    tc.tile_pool(name="qk_ps", bufs=5, space="PSUM") as qk_ps,
                    tc.tile_pool(name="v_ps", bufs=3, space="PSUM") as v_ps,
                ):
                    if w_const:
                        w_sb = w_const_sb
                    else:
                        w_sb = []
                        for k in range(KD):
                            wt = wpool.tile([128, 1536], io_dt, tag="w")
                            nc.sync.dma_start(
                                out=wt[:], in_=wqkvT[k * 128 : (k + 1) * 128, :]
                            )
                            w_sb.append(wt)
                    copy_engines = [
                        lambda o, i: nc.scalar.copy(o, i),
                        lambda o, i: nc.vector.tensor_copy(o, i),
                    ]
                    ncopy = 0
                    for nb in range(NBLK):
                        x_sb = []
                        for k in range(KD):
                            xt = xpool.tile([128, NB], io_dt, tag="x")
                            nc.sync.dma_start(
                                out=xt[:],
                                in_=xT[k * 128 : (k + 1) * 128, nb * NB : (nb + 1) * NB],
                            )
                            x_sb.append(xt)
                        if nb == 0:
                            bias_tiles[0] = load_bias(0)
                            bias_tiles[1] = load_bias(1)
                        for e in range(8):
                            ps = qk_ps.tile([128, NB], f32, tag="qk")
                            for k in range(KD):
                                nc.tensor.matmul(
                                    ps[:],
                                    w_sb[k][:, e * 128 : (e + 1) * 128],
                                    x_sb[k][:],
                                    start=(k == 0),
                                    stop=(k == KD - 1),
                                )
                            dst = qT[e] if e < 4 else kT[e - 4]
                            eng = copy_engines[ncopy % 2]
                            ncopy += 1
                            eng(dst[:, nb * NB : (nb + 1) * NB], ps[:])
                        for mi in range(4):
                            m = nb * 4 + mi
                            ps = v_ps.tile([128, NB], f32, tag="v")
                            for k in range(KD):
                                nc.tensor.matmul(
                                    ps[:],
                                    x_sb[k][:, mi * 128 : (mi + 1) * 128],
                                    w_sb[k][:, 1024:1536],
                                    start=(k == 0),
                                    stop=(k == KD - 1),
                                )
                            eng = copy_engines[ncopy % 2]
                            ncopy += 1
                            eng(
                                V[m][:].rearrange("p (h c) -> p h c", c=65)[:, :, 0:64],
                                ps[:].rearrange("p (h c) -> p h c", c=64),
                            )

                # phase 2: attention + projection
                with (
                    tc.tile_pool(name="esb", bufs=8) as e_pool,
                    tc.tile_pool(name="opair", bufs=8) as o_pool,
                    tc.tile_pool(name="ysb", bufs=3) as y_pool,
                    tc.tile_pool(name="nrm", bufs=4) as nrm_pool,
                    tc.tile_pool(name="l_ps", bufs=2, space="PSUM") as l_ps,
                    tc.tile_pool(name="o_ps", bufs=3, space="PSUM") as o_psp,
                    tc.tile_pool(name="bc_ps", bufs=1, space="PSUM") as bc_psp,
                ):
                    o_pairs = [None] * 4
                    pend = []
                    norm_pend = []

                    def recip_stage(o_tiles):
                        recips = []
                        for hh in range(2):
                            recip = nrm_pool.tile([1, NB], f32r, tag="recip")
                            with nc.allow_low_precision(
                                reason="f32r recip feeds f32r matmul"
                            ):
                                nc.vector.reciprocal(
                                    recip[:], o_tiles[hh][64:65, :]
                                )
                            recips.append(recip)
                        return recips

                    def norm_stage(j, o_tiles, recips):
                        o_pair = o_pool.tile([128, NB], io_dt, tag="op")
                        o_pairs[j] = o_pair
                        for hh in range(2):
                            off = hh * 64
                            bc = bc_psp.tile([64, NB], f32, tag="bc")
                            nc.tensor.matmul(
                                bc[:],
                                ones_row[:, 0:64],
                                recips[hh][:],
                                start=True,
                                stop=True,
                            )
                            bcs = nrm_pool.tile([64, NB], f32, tag="bcs")
                            if bcs_mode == "act":
                                nc.scalar.copy(bcs[:], bc[:])
                            else:
                                nc.vector.tensor_copy(bcs[:], bc[:])
                            nc.vector.tensor_tensor(
                                o_pair[off : off + 64, :],
                                o_tiles[hh][0:64, :],
                                bcs[:],
                                ALU.mult,
                            )

                    def flush_norms():
                        while norm_pend:
                            norm_stage(*norm_pend.pop(0))

                    def emit_pv(o_tiles, j, m, et):
                        for hh in range(2):
                            h = 2 * j + hh
                            nc.tensor.matmul(
                                o_tiles[hh][:],
                                V[m][:, h * 65 : (h + 1) * 65],
                                et[:, hh * NB : (hh + 1) * NB],
                                start=(m == 0),
                                stop=(m == MT - 1),
                            )
                        if m == MT - 1:
                            norm_pend.append((j, o_tiles, recip_stage(o_tiles)))
                            if not norm_delay:
                                flush_norms()

                    def alloc_o(nb, j):
                        return [
                            o_psp.tile([65, NB], f32, tag="ops", name=f"o{nb}_{j}_{hh}")
                            for hh in range(2)
                        ]

                    def emit_step(o_tiles, j, m, bias_sb, nb):
                        biased = j < biased_pairs
                        lp = l_ps.tile([128, 2 * NB], f32, tag="lp")
                        for hh in range(2):
                            off = hh * 64
                            i = 2 * j + hh
                            on_dve = biased and i < dve_bias_heads
                            nc.tensor.matmul(
                                lp[:, hh * NB : (hh + 1) * NB],
                                kT[j][off : off + 64, m * 128 : (m + 1) * 128],
                                qT[j][off : off + 64, nb * NB : (nb + 1) * NB],
                                start=True,
                                stop=(not biased) or on_dve,
                                tile_position=(off, 0),
                            )
                        if biased:
                            for hh in range(2):
                                i = 2 * j + hh
                                if i < dve_bias_heads:
                                    nc.vector.scalar_tensor_tensor(
                                        lp[:, hh * NB : (hh + 1) * NB],
                                        bias_sb[m][:].bitcast(f32),
                                        beta8_sb[:, i : i + 1],
                                        lp[:, hh * NB : (hh + 1) * NB],
                                        op0=ALU.mult,
                                        op1=ALU.add,
                                    )
                                else:
                                    nc.tensor.matmul(
                                        lp[:, hh * NB : (hh + 1) * NB],
                                        ident_sb[:, i * 128 : (i + 1) * 128],
                                        bias_sb[m][:],
                                        start=False,
                                        stop=True,
                                    )
                        et = e_pool.tile([128, 2 * NB], qk_dt, tag="e")
                        nc.scalar.activation(et[:], lp[:], AF.Exp, scale=SCALE)
                        pend.append((o_tiles, j, m, et))
                        if len(pend) > 2:
                            emit_pv(*pend.pop(0))
                            if norm_pend and len(pend) > 1:
                                flush_norms()

                    primed = None
                    for nb in range(NBLK):
                        bias_sb = bias_tiles.pop(nb)
                        for j in range(4):
                            if j == 0 and primed is not None:
                                o_tiles, m_start = primed
                                primed = None
                            else:
                                o_tiles = alloc_o(nb, j)
                                m_start = 0
                            for m in range(m_start, MT):
                                emit_step(o_tiles, j, m, bias_sb, nb)
                            if j == 1 and nb + 2 < NBLK:
                                bias_tiles[nb + 2] = load_bias(nb + 2)
                        while pend:
                            emit_pv(*pend.pop(0))
                        if nb + 1 < NBLK:
                            nxt_bias = bias_tiles[nb + 1]
                            o_t = alloc_o(nb + 1, 0)
                            emit_step(o_t, 0, 0, nxt_bias, nb + 1)
                            emit_step(o_t, 0, 1, nxt_bias, nb + 1)
                            primed = (o_t, 2)
                        flush_norms()

                        for pair in ((0, 1), (2, 3)):
                            yps = {}
                            nj = 3 if proj_split else 4
                            for nt in pair:
                                yp = l_ps.tile([128, 2 * NB], f32, tag="lp")
                                yps[nt] = yp
                                for db in range(2):
                                    for j in range(nj):
                                        nc.tensor.matmul(
                                            yp[:, db * NB : (db + 1) * NB],
                                            o_pairs[j][:, nt * 128 : (nt + 1) * 128],
                                            wproj_sb[j][:, db * NB : (db + 1) * NB],
                                            start=(j == 0),
                                            stop=(j == 3),
                                        )
                            for nt in pair:
                                yp = yps[nt]
                                if proj_split:
                                    for db in range(2):
                                        nc.tensor.matmul(
                                            yp[:, db * NB : (db + 1) * NB],
                                            o_pairs[3][:, nt * 128 : (nt + 1) * 128],
                                            wproj_sb[3][:, db * NB : (db + 1) * NB],
                                            start=False,
                                            stop=True,
                                        )
                                ysb = y_pool.tile([128, 2 * NB], f32, tag="y")
                                if ysb_mode == "act":
                                    nc.scalar.copy(ysb[:], yp[:])
                                else:
                                    nc.vector.tensor_copy(ysb[:], yp[:])
                                nc.sync.dma_start(
                                    out=y[
                                        nb * NB + nt * 128 : nb * NB + (nt + 1) * 128, :
                                    ],
                                    in_=ysb[:],
                                )

                bias_ctx.__exit__(None, None, None)

            if reps == 1:
                body()
            else:
                with tc.For_i(0, reps, 1):
                    body()
            if timing:
                dum = const_pool.tile([1, 4], f32, tag="dum")
                nc.vector.memset(dum[:], 0.0)
                nc.sync.dma_start(out=ydum[:], in_=dum[:])

    _split_multi_waits(nc)
    nc.finalize()
    return nc


def make_core_inputs(x, attn_bias, Wqkv, Wproj, beta, core_id, all_bf16=False):
    import ml_dtypes
    io_np = ml_dtypes.bfloat16 if all_bf16 else np.float32
    b = core_id // 2
    parity = core_id % 2
    hlist = list(range(parity, 16, 2))
    rows = np.concatenate([np.arange(h * 64, (h + 1) * 64) for h in hlist])
    wqkvT = np.ascontiguousarray(
        np.concatenate([Wqkv[rows], Wqkv[D + rows], Wqkv[2 * D + rows]], 0).T
    )
    wprojT = np.ascontiguousarray(Wproj.T[rows])
    beta4 = np.asarray(beta).reshape(-1)[hlist[:4]].astype(np.float32)
    ident4 = np.zeros((128, 512), dtype=np.float32)
    eye = np.eye(128, dtype=np.float32)
    for i in range(4):
        # exp is taken with scale=0.125, so pre-scale the bias by 8*beta
        ident4[:, i * 128 : (i + 1) * 128] = eye * (8.0 * beta4[i])
    return {
        "xT": np.ascontiguousarray(x[b].T).astype(io_np),
        "wqkvT": wqkvT.astype(io_np),
        "wprojT": wprojT.astype(io_np),
        "biasT": np.ascontiguousarray(attn_bias[b, 0].T).astype(io_np),
        "ident4": ident4.astype(io_np),
        "beta8": np.tile(8.0 * beta4.reshape(1, 4), (128, 1)).astype(np.float32),
    }


_NC_CACHE = {}

# Graded configuration (HW A/Bs in test.py): bf16 matmul operands
# everywhere (fp32 PSUM), QKV weights resident across reps, split
# out-proj accumulation, inline normalize emission.
# Measured: 553.8us/iter, absmax_rel=1.009e-02 (gate 2e-2).
CONFIG = dict(attn_bf16=True, all_bf16=True, w_const=True,
              proj_split=True, norm_delay=True)


def kernel(x, attn_bias, Wqkv, Wproj, beta):
    from concourse.bass_utils import run_bass_kernel_spmd

    x = np.asarray(x, dtype=np.float32)
    attn_bias = np.asarray(attn_bias, dtype=np.float32)
    Wqkv = np.asarray(Wqkv, dtype=np.float32)
    Wproj = np.asarray(Wproj, dtype=np.float32)
    beta = np.asarray(beta, dtype=np.float32)

    key = tuple(sorted(CONFIG.items()))
    if key not in _NC_CACHE:
        _NC_CACHE[key] = build_nc(reps=1, **CONFIG)
    nc = _NC_CACHE[key]

    in_maps = [
        make_core_inputs(
            x, attn_bias, Wqkv, Wproj, beta, core, all_bf16=CONFIG["all_bf16"]
        )
        for core in range(8)
    ]
    res = run_bass_kernel_spmd(nc, in_maps, core_ids=list(range(8)))
    out = np.zeros((4, N, D), dtype=np.float32)
    for core_id in range(8):
        out[core_id // 2] += res.results[core_id]["y"]
    return out
